# revision 48
# baseline (speedup 1.0000x reference)
"""Trainium2 Bass kernel for nn_Attention_19404662243470.

Sharding: 8 cores = (batch 2) x (heads 4). Each core computes the full
attention pipeline for its (b, h) pair in transposed layout [d, n]; the
final pointwise conv partials are ReduceScattered (2 ops) within each
batch's 4-core group, and LayerNorm2d runs on each core's position shard.

Key layout/speed choices:
 - everything bf16 on the PE paths (FWL weight loads, halved DMA bytes);
   fp32 only inside PSUM and the LN scalar chains.
 - q/k/v come out of the pointwise conv directly as [d, n] ("T layout").
 - softmax runs without max-subtraction; denominator falls out of the AV
   matmul via an appended ones-row in V.
 - exp is split across engines: ACT does 6 of 8 j-groups per chunk,
   DVE does 2 via the Schraudolph int-trick (x*A+B -> int32, whose top
   16 bits ARE the bf16 exp); the AV matmul reads the int32 tile through
   a stride-2 bf16 view.
 - out-LN uses scale invariance: LN(num/den + v) == LN(num + den*v), so
   no reciprocal is ever computed.
 - per-head LN stats in stage A are batched: chunk c's mean/E[x^2] rows
   land on PSUM partition c via a column-selecting stationary, so the
   whole LN chain runs on [8, 512] tiles instead of [1, 4096].
 - q/k halves are mirrored to partitions 64-127 with SBUF->SBUF DMA
   (row packing doubles S^T throughput; contraction is only 64 deep).
"""

import numpy as np

import concourse.bass as bass
import concourse.tile as tile
from concourse import bacc, mybir
from concourse.bass_utils import run_bass_kernel_spmd

dt = mybir.dt
AF = mybir.ActivationFunctionType
OP = mybir.AluOpType

B, DIM, Hs, Ws = 2, 128, 64, 64
HEADS, DH = 4, 64
N = Hs * Ws  # 4096
EPS = 1e-6
IC = 512  # i-chunk width
NIC = N // IC  # 8
JB = 128  # j-block
NJB = N // JB  # 32
NCH = N // 128  # 32
GR = Hs + 2  # 66 grid rows
GC = Hs + 4  # 68 grid cols (interior at col 2 -> 4B-aligned bf16 rows)

# Schraudolph fast-exp: exp(x/8) ~= bf16_bits(int32(x*EXA + EXB) >> 16)
EXA = float((2.0**23) / np.log(2.0) / 8.0)
EXB = float(127 * 2**23 - 335000.0)

_TABLES_PATCHED = False


def _patch_act_tables():
    """Restrict Exp/Ln to the natural_log_exp_and_others set so the ACT
    table never reloads between the softmax Exp stream and the LN-chain
    Ln/Exp pairs (a reload costs ~2.7us and stalls the exp feed)."""
    global _TABLES_PATCHED
    if _TABLES_PATCHED:
        return
    from concourse import bacc as _bacc_mod

    orig = _bacc_mod.get_activation_tables

    def patched(arch):
        tabs = dict(orig(arch))
        keep = {mybir.ActivationFunctionType.Exp, mybir.ActivationFunctionType.Ln}
        return {
            name: (fns if name == "natural_log_exp_and_others" else fns - keep)
            for name, fns in tabs.items()
        }

    _bacc_mod.get_activation_tables = patched
    _TABLES_PATCHED = True


def _build():
    _patch_act_tables()
    nc = bacc.Bacc()

    def par(name, shape, dtyp=dt.float32):
        return nc.declare_dram_parameter(name, list(shape), dtyp, isOutput=False)

    x = par("x", [DIM, GR * GC], dt.bfloat16)  # pre-padded grid layout
    dwpos = par("dwpos", [DIM, N], dt.bfloat16)  # dw3x3(pos), CPU-precomputed
    qdiags = par("qdiags", [DIM, 9 * DIM], dt.bfloat16)
    pwq = par("pwq", [DIM, DH], dt.bfloat16)
    pwk = par("pwk", [DIM, DH], dt.bfloat16)
    pwv = par("pwv", [DIM, DH], dt.bfloat16)
    o8sel = par("o8sel", [DH, 8 * 8], dt.bfloat16)  # slice c: [64,8], col c=1/64
    o8self = par("o8self", [DH, 8 * 8], dt.float32r)  # f32r twin (for f32r moving)
    w8q = par("w8q", [8, 8 * DH], dt.float32r)  # slice c: [8,64], row c=nq_w
    w8k = par("w8k", [8, 8 * DH], dt.float32r)
    lnqb = par("lnqb", [DH, 1])
    lnkb = par("lnkb", [DH, 1])
    lnow = par("lnow", [1, DH], dt.float32r)
    lnob = par("lnob", [DH, 1])
    odiags = par("odiags", [DH, 9 * DH], dt.bfloat16)
    opw = par("opw", [DH, DIM], dt.bfloat16)
    ln2w = par("ln2w", [1, DIM])
    ln2b = par("ln2b", [1, DIM])
    o64hd = par("o64h", [DH, 1], dt.float32r)
    onesrd = par("onesr", [1, DH], dt.float32r)
    out_ext = nc.declare_dram_parameter("out", [N // 4, DIM], dt.float32, isOutput=True)

    rs_in = nc.dram_tensor("rs_in", [N, DIM], dt.bfloat16)
    rs_out = nc.dram_tensor("rs_out", [N // 4, DIM], dt.bfloat16)

    with (
        nc.allow_low_precision(reason="bf16 compute by design"),
        tile.TileContext(nc) as tc,
        tc.tile_pool(name="main", bufs=1) as main,
        tc.tile_pool(name="tmp2", bufs=2) as tmp2,
    ):
        # ---- input DMAs first: they gate the depthwise conv ----
        Xg = main.tile([DIM, GR * GC], dt.bfloat16)
        DWPOS = main.tile([DIM, N], dt.bfloat16)
        qdg = main.tile([DIM, 9, DIM], dt.bfloat16)
        HGC = (GR // 2) * GC
        nc.sync.dma_start(out=Xg[:, 0:HGC], in_=x[:, 0:HGC])
        nc.scalar.dma_start(out=Xg[:, HGC:], in_=x[:, HGC:])
        nc.gpsimd.dma_start(
            out=qdg, in_=qdiags[:, :].rearrange("p (t c) -> p t c", t=9)
        )
        nc.gpsimd.dma_start(out=DWPOS, in_=dwpos[:, :])
        Xg = Xg.rearrange("p (r c) -> p r c", c=GC)

        # ---- persistent SBUF tiles ----
        QL = main.tile([128, N], dt.bfloat16)  # LN'd q, duplicated on both halves
        KL = main.tile([128, N], dt.bfloat16)
        QRW = main.tile([DH, N], dt.bfloat16)  # raw q (pre-LN); applied per-chunk
        SCB8q = main.tile([8, 2 * IC], dt.float32r)  # q-LN rs | mu*rs
        VT = main.tile([DH, N], dt.bfloat16)  # v^T for the skip connection
        V = main.tile([128, NCH, DH + 1], dt.bfloat16)
        SC = main.tile([1, 2 * N], dt.float32)  # attention out-LN: mu | E2
        SCB = main.tile([1, 2 * N], dt.float32r)  # rs | mu*rs (matmul-ready)
        Og = main.tile([DH, GR, GC], dt.bfloat16)  # padded out-LN grid
        odg = main.tile([DH, 9, DH], dt.bfloat16)
        nc.scalar.dma_start(
            out=odg, in_=odiags[:, :].rearrange("p (t c) -> p t c", t=9)
        )
        opw_t = main.tile([DH, DIM], dt.bfloat16)
        nc.scalar.dma_start(out=opw_t, in_=opw[:, :])
        DWO = main.tile([DH, N], dt.bfloat16)
        o64h = main.tile([DH, 1], dt.float32r)
        nc.sync.dma_start(out=o64h, in_=o64hd[:, :])
        o8sel_t = main.tile([DH, 8, 8], dt.bfloat16)
        nc.sync.dma_start(out=o8sel_t, in_=o8sel[:, :].rearrange("p (c e) -> p c e", c=8))
        o8self_t = main.tile([DH, 8, 8], dt.float32r)
        nc.sync.dma_start(
            out=o8self_t, in_=o8self[:, :].rearrange("p (c e) -> p c e", c=8)
        )
        w8q_t = main.tile([8, 8, DH], dt.float32r)
        nc.sync.dma_start(out=w8q_t, in_=w8q[:, :].rearrange("p (c e) -> p c e", c=8))
        w8k_t = main.tile([8, 8, DH], dt.float32r)
        nc.sync.dma_start(out=w8k_t, in_=w8k[:, :].rearrange("p (c e) -> p c e", c=8))
        lnqb_t = main.tile([DH, 1], dt.float32)
        lnkb_t = main.tile([DH, 1], dt.float32)
        lnob_t = main.tile([DH, 1], dt.float32)
        nc.sync.dma_start(out=lnqb_t, in_=lnqb[:, :])
        nc.sync.dma_start(out=lnkb_t, in_=lnkb[:, :])
        nc.sync.dma_start(out=lnob_t, in_=lnob[:, :])
        lnow_t = main.tile([1, DH], dt.float32r)
        nc.sync.dma_start(out=lnow_t, in_=lnow[:, :])
        onesr = main.tile([1, DH], dt.float32r)
        nc.sync.dma_start(out=onesr, in_=onesrd[:, :])
        epsP = main.tile([128, 1], dt.float32)
        nc.vector.memset(epsP, EPS)
        nc.vector.memset(V, 1.0)
        nc.vector.memset(Og, 0.0)

        # ============ Stage A1: pos + depthwise ============
        with tc.tile_pool(name="stageA", bufs=1) as pA:
            psA1cm = tc.tile_pool(name="psA1", bufs=2, space="PSUM")
            psA1 = psA1cm.__enter__()
            pADWcm = tc.tile_pool(name="pADW", bufs=1)
            pADW = pADWcm.__enter__()
            pwq_t = pA.tile([DIM, DH], dt.bfloat16)
            pwk_t = pA.tile([DIM, DH], dt.bfloat16)
            pwv_t = pA.tile([DIM, DH], dt.bfloat16)
            nc.sync.dma_start(out=pwq_t, in_=pwq[:, :])
            nc.sync.dma_start(out=pwk_t, in_=pwk[:, :])
            nc.sync.dma_start(out=pwv_t, in_=pwv[:, :])

            # depthwise 3x3 via 9 accumulated diag matmuls (+ dw(pos) bias)
            Yr = pA.tile([DIM, N], dt.bfloat16)
            for c in range(NIC):
                dwp = psA1.tile([DIM, IC], dt.float32, tag="dw")
                r0 = c * 8
                t = 0
                for di in range(3):
                    for dj in range(3):
                        nc.tensor.matmul(
                            dwp,
                            qdg[:, t, :],
                            Xg[:, r0 + di : r0 + di + 8, 1 + dj : 1 + dj + Ws],
                            start=(t == 0),
                            stop=(t == 8),
                        )
                        t += 1
                nc.vector.tensor_add(
                    out=Yr[:, c * IC : (c + 1) * IC],
                    in0=DWPOS[:, c * IC : (c + 1) * IC],
                    in1=dwp,
                )

            psA1cm.__exit__(None, None, None)
            pADWcm.__exit__(None, None, None)
            # ============ Stage A2: pointwise + q/k LN + v ============
            with tc.tile_pool(name="psA2", bufs=1, space="PSUM") as psA2:
                KRAW = pA.tile([DH, N], dt.bfloat16)
                SC8q = pA.tile([8, 2 * IC], dt.float32)
                SC8k = pA.tile([8, 2 * IC], dt.float32)
                SCB8k = pA.tile([8, 2 * IC], dt.float32r)

                def ptwise(dst, pw_t):
                    for c in range(NIC):
                        qp = psA2.tile([DH, IC], dt.float32, tag="qp", bufs=2)
                        nc.tensor.matmul(
                            qp, pw_t, Yr[:, c * IC : (c + 1) * IC], start=True, stop=True
                        )
                        # alternate copy engines: ACT and DVE drain PSUM in
                        # parallel so the stats matmuls are gated half as long
                        if c % 2 == 0:
                            nc.scalar.copy(out=dst[:, c * IC : (c + 1) * IC], in_=qp)
                        else:
                            nc.vector.tensor_copy(
                                out=dst[:, c * IC : (c + 1) * IC], in_=qp
                            )

                def stats8(src, sc8):
                    """mu and E[x^2] of every chunk, chunk c on partition c."""
                    mu8 = psA2.tile([8, IC], dt.float32, tag="mu8", bufs=1)
                    e28 = psA2.tile([8, IC], dt.float32, tag="e28", bufs=1)
                    for c in range(NIC):
                        sq = tmp2.tile([DH, IC], dt.float32r, tag="sq", bufs=2)
                        src_c = src[:, c * IC : (c + 1) * IC]
                        nc.vector.tensor_mul(out=sq, in0=src_c, in1=src_c)
                        nc.tensor.matmul(
                            mu8,
                            o8sel_t[:, c, :],
                            src_c,
                            start=(c == 0),
                            stop=(c == NIC - 1),
                            skip_group_check=True,
                        )
                        nc.tensor.matmul(
                            e28,
                            o8self_t[:, c, :],
                            sq,
                            start=(c == 0),
                            stop=(c == NIC - 1),
                            skip_group_check=True,
                        )
                    nc.scalar.copy(out=sc8[:, 0:IC], in_=mu8)
                    nc.scalar.copy(out=sc8[:, IC : 2 * IC], in_=e28)

                def chain8(sc8, scb8):
                    mu = sc8[:, 0:IC]
                    e2 = sc8[:, IC : 2 * IC]
                    rs = scb8[:, 0:IC]
                    mrs = scb8[:, IC : 2 * IC]
                    nc.vector.scalar_tensor_tensor(
                        out=mrs, in0=mu, scalar=-1.0, in1=mu, op0=OP.mult, op1=OP.mult
                    )
                    nc.vector.tensor_add(out=e2, in0=e2, in1=mrs)
                    nc.scalar.activation(out=e2, in_=e2, func=AF.Ln, bias=epsP[0:8, :])
                    nc.scalar.activation(out=rs, in_=e2, func=AF.Exp, scale=-0.5)
                    nc.vector.tensor_mul(out=mrs, in0=mu, in1=rs)

                def apply8(src, scb8, w8_t, b_t, dst):
                    for c in range(NIC):
                        bcA = psA2.tile([DH, IC], dt.float32, tag="bc", bufs=3)
                        nc.tensor.matmul(
                            bcA, w8_t[:, c, :], scb8[:, 0:IC], start=True, stop=True
                        )
                        bcB = psA2.tile([DH, IC], dt.float32, tag="bc", bufs=3)
                        nc.tensor.matmul(
                            bcB,
                            w8_t[:, c, :],
                            scb8[:, IC : 2 * IC],
                            start=True,
                            stop=True,
                        )
                        T = tmp2.tile([DH, IC], dt.bfloat16, tag="T")
                        nc.vector.tensor_mul(
                            out=T, in0=src[:, c * IC : (c + 1) * IC], in1=bcA
                        )
                        nc.vector.scalar_tensor_tensor(
                            out=dst[0:DH, c * IC : (c + 1) * IC],
                            in0=T,
                            scalar=b_t,
                            in1=bcB,
                            op0=OP.add,
                            op1=OP.subtract,
                        )

                def vbuild(lo, hi):
                    """V ([pos, dh] layout) + VT ([dh, pos]) for chunks [lo, hi);
                    pure-PE filler issued under the LN chains to keep HAM warm."""
                    for g in range(lo, hi):
                        vp = psA2.tile([128, 4 * DH], dt.float32, tag="vp", bufs=1)
                        for t in range(4):
                            ch = 4 * g + t
                            nc.tensor.matmul(
                                vp[:, t * DH : (t + 1) * DH],
                                Yr[:, ch * 128 : (ch + 1) * 128],
                                pwv_t,
                                start=True,
                                stop=True,
                            )
                        if g % 2 == 0:
                            nc.scalar.copy(
                                out=V[:, 4 * g : 4 * g + 4, 0:DH],
                                in_=vp.rearrange("p (t d) -> p t d", t=4),
                            )
                        else:
                            nc.vector.tensor_copy(
                                out=V[:, 4 * g : 4 * g + 4, 0:DH],
                                in_=vp.rearrange("p (t d) -> p t d", t=4),
                            )
                        qp = psA2.tile([DH, IC], dt.float32, tag="qp", bufs=2)
                        nc.tensor.matmul(
                            qp, pwv_t, Yr[:, g * IC : (g + 1) * IC], start=True, stop=True
                        )
                        if g % 2 == 1:
                            nc.scalar.copy(out=VT[:, g * IC : (g + 1) * IC], in_=qp)
                        else:
                            nc.vector.tensor_copy(
                                out=VT[:, g * IC : (g + 1) * IC], in_=qp
                            )

                # k first; its chain overlaps v-builds + q pointwise on the PE.
                # q's LN apply is deferred into the attention loop (chunk-wise)
                # so the PE never idles long enough for HAM to re-throttle.
                ptwise(KRAW, pwk_t)
                stats8(KRAW, SC8k)
                chain8(SC8k, SCB8k)
                vbuild(0, NIC // 2)
                ptwise(QRW, pwq_t)
                stats8(QRW, SC8q)

                apply8(KRAW, SCB8k, w8k_t, lnkb_t, KL)
                nc.scalar.dma_start(out=KL[DH:128, :], in_=KL[0:DH, :])

                chain8(SC8q, SCB8q)
                vbuild(NIC // 2, NIC)

        # ============ Stage B: attention with inline out-LN ============
        with tc.tile_pool(name="psB", bufs=1, space="PSUM") as psB, tc.tile_pool(
            name="sbB", bufs=3
        ) as sbB:
            NG = NJB // 2  # 16 pair-groups per chunk
            DVE_G = (3, 7, 11, 15)  # groups whose exp runs on DVE (Schraudolph)
            pending_tail = []

            def apply_q(c):
                """q-LN apply for chunk c, pipelined inside the attention loop."""
                bcA = psB.tile([DH, IC], dt.float32, tag="st", bufs=2)
                nc.tensor.matmul(
                    bcA, w8q_t[:, c, :], SCB8q[:, 0:IC], start=True, stop=True
                )
                bcB = psB.tile([DH, IC], dt.float32, tag="st", bufs=2)
                nc.tensor.matmul(
                    bcB, w8q_t[:, c, :], SCB8q[:, IC : 2 * IC], start=True, stop=True
                )
                T = tmp2.tile([DH, IC], dt.bfloat16, tag="Tq")
                nc.vector.tensor_mul(
                    out=T, in0=QRW[:, c * IC : (c + 1) * IC], in1=bcA
                )
                nc.vector.scalar_tensor_tensor(
                    out=QL[0:DH, c * IC : (c + 1) * IC],
                    in0=T,
                    scalar=lnqb_t,
                    in1=bcB,
                    op0=OP.add,
                    op1=OP.subtract,
                )
                nc.scalar.dma_start(
                    out=QL[DH:128, c * IC : (c + 1) * IC],
                    in_=QL[0:DH, c * IC : (c + 1) * IC],
                )

            def attention_block(c):
                avp = psB.tile([DH + 1, IC], dt.float32, tag="avp", bufs=1)
                stgs = {}
                Es = {}

                def issue_st(g):
                    stg = psB.tile([128, 2 * IC], dt.float32, tag="stg", bufs=2)
                    j0 = 2 * g * JB
                    nc.tensor.matmul(
                        stg[:, 0:IC],
                        KL[0:DH, j0 : j0 + JB],
                        QL[0:DH, c * IC : (c + 1) * IC],
                        start=True,
                        stop=True,
                    )
                    nc.tensor.matmul(
                        stg[:, IC : 2 * IC],
                        KL[DH:128, j0 + JB : j0 + 2 * JB],
                        QL[DH:128, c * IC : (c + 1) * IC],
                        start=True,
                        stop=True,
                    )
                    stgs[g] = stg

                def issue_exp(g):
                    if g in DVE_G:
                        EI = sbB.tile([128, 2 * IC], dt.int32, tag="EI", bufs=2)
                        nc.vector.tensor_scalar(
                            out=EI,
                            in0=stgs.pop(g),
                            scalar1=EXA,
                            scalar2=EXB,
                            op0=OP.mult,
                            op1=OP.add,
                        )
                        Es[g] = EI.bitcast(dt.bfloat16).rearrange(
                            "p (a two) -> p a two", two=2
                        )
                    else:
                        E = sbB.tile([128, 2 * IC], dt.bfloat16, tag="E")
                        nc.scalar.activation(
                            out=E, in_=stgs.pop(g), func=AF.Exp, scale=float(DH**-0.5)
                        )
                        Es[g] = E

                def issue_av(g):
                    E = Es.pop(g)
                    for t in range(2):
                        jb = 2 * g + t
                        if g in DVE_G:
                            rhs = E[:, t * IC : (t + 1) * IC, 1:2]
                        else:
                            rhs = E[:, t * IC : (t + 1) * IC]
                        nc.tensor.matmul(
                            avp,
                            V[:, jb, :],
                            rhs,
                            start=(jb == 0),
                            stop=(jb == NJB - 1),
                            skip_group_check=True,
                        )

                issue_st(0)
                issue_exp(0)
                for g in range(1, NG):
                    issue_st(g)
                    issue_exp(g)
                    issue_av(g - 1)
                issue_av(NG - 1)

                # park numerator+denominator info quickly to free avp:
                # DEN row copy; numerator stays in avp until tail (bufs=1 ok:
                # tail runs during the NEXT chunk's matmuls, before its avp use)
                DEN = sbB.tile([1, IC], dt.float32r, tag="DEN", bufs=2)
                nc.vector.tensor_copy(out=DEN, in_=avp[DH : DH + 1, :])
                Tn = sbB.tile([DH, IC], dt.float32, tag="Tn", bufs=2)
                nc.vector.tensor_copy(out=Tn, in_=avp[0:DH, :])
                return DEN, Tn

            def stats_mms(psp, src_ap, c):
                sq = tmp2.tile([DH, IC], dt.float32r, tag="sqo", bufs=1)
                nc.gpsimd.tensor_mul(out=sq, in0=src_ap, in1=src_ap)
                smu = psp.tile([1, IC], dt.float32, tag="st", bufs=2)
                nc.tensor.matmul(smu, o64h, src_ap, start=True, stop=True)
                nc.vector.tensor_copy(out=SC[:, c * IC : (c + 1) * IC], in_=smu)
                se2 = psp.tile([1, IC], dt.float32, tag="st", bufs=2)
                nc.tensor.matmul(se2, o64h, sq, start=True, stop=True)
                nc.vector.tensor_copy(out=SC[:, N + c * IC : N + (c + 1) * IC], in_=se2)

            def ln_chain(lo, hi):
                mu = SC[:, lo:hi]
                e2 = SC[:, N + lo : N + hi]
                mrs = SCB[:, N + lo : N + hi]
                rs = SCB[:, lo:hi]
                nc.vector.scalar_tensor_tensor(
                    out=mrs, in0=mu, scalar=-1.0, in1=mu, op0=OP.mult, op1=OP.mult
                )
                nc.vector.tensor_add(out=e2, in0=e2, in1=mrs)
                nc.scalar.activation(out=e2, in_=e2, func=AF.Ln, bias=epsP[0:1, :])
                nc.scalar.activation(out=rs, in_=e2, func=AF.Exp, scale=-0.5)
                nc.vector.tensor_mul(out=mrs, in0=mu, in1=rs)

            def tail_block(c, DEN, Tn):
                # scale-invariant skip: OSc = num + den*v (LN output matches
                # LN(num/den + v) because LN normalizes per-position scale)
                bcD = psB.tile([DH, IC], dt.float32, tag="st", bufs=2)
                nc.tensor.matmul(bcD, onesr, DEN, start=True, stop=True)
                OSc = sbB.tile([DH, IC], dt.float32r, tag="OS", bufs=2)
                nc.vector.tensor_mul(
                    out=OSc, in0=VT[:, c * IC : (c + 1) * IC], in1=bcD
                )
                nc.vector.tensor_add(out=OSc, in0=OSc, in1=Tn)
                stats_mms(psB, OSc[:, :], c)
                ln_chain(c * IC, (c + 1) * IC)
                bcA = psB.tile([DH, IC], dt.float32, tag="st", bufs=2)
                nc.tensor.matmul(
                    bcA, lnow_t, SCB[:, c * IC : (c + 1) * IC], start=True, stop=True
                )
                bcB = psB.tile([DH, IC], dt.float32, tag="st", bufs=2)
                nc.tensor.matmul(
                    bcB,
                    lnow_t,
                    SCB[:, N + c * IC : N + (c + 1) * IC],
                    start=True,
                    stop=True,
                )
                T = tmp2.tile([DH, IC], dt.float32, tag="T")
                nc.vector.tensor_mul(out=T, in0=OSc, in1=bcA)
                r0 = c * 8
                nc.vector.scalar_tensor_tensor(
                    out=Og[:, 1 + r0 : 9 + r0, 2 : 2 + Ws],
                    in0=T.rearrange("p (a b) -> p a b", b=Ws),
                    scalar=lnob_t,
                    in1=bcB.rearrange("p (a b) -> p a b", b=Ws),
                    op0=OP.add,
                    op1=OP.subtract,
                )

            def dw_chunk(c):
                dwpf = psB.tile([128, IC], dt.float32, tag="dwpp", bufs=1)
                dwp = dwpf[0:DH, :]
                r0 = c * 8
                t = 0
                for di in range(3):
                    for dj in range(3):
                        nc.tensor.matmul(
                            dwp,
                            odg[:, t, :],
                            Og[:, r0 + di : r0 + di + 8, 1 + dj : 1 + dj + Ws],
                            start=(t == 0),
                            stop=(t == 8),
                        )
                        t += 1
                nc.vector.tensor_copy(out=DWO[:, c * IC : (c + 1) * IC], in_=dwp)
                pp = psB.tile([128, 4 * DIM], dt.float32, tag="dwpp", bufs=1)
                for t in range(4):
                    ch = 4 * c + t
                    nc.tensor.matmul(
                        pp[:, t * DIM : (t + 1) * DIM],
                        DWO[:, ch * 128 : (ch + 1) * 128],
                        opw_t,
                        start=True,
                        stop=True,
                    )
                PP = tmp2.tile([128, 4 * DIM], dt.bfloat16, tag="PP")
                nc.vector.tensor_copy(out=PP, in_=pp)
                for t in range(4):
                    ch = 4 * c + t
                    nc.sync.dma_start(
                        out=rs_in[ch * 128 : (ch + 1) * 128, :],
                        in_=PP[:, t * DIM : (t + 1) * DIM],
                    )
                if c % 2 == 1:
                    p = c // 2
                    nc.gpsimd.collective_compute(
                        "ReduceScatter",
                        OP.add,
                        replica_groups=[[0, 1, 2, 3], [4, 5, 6, 7]],
                        ins=[rs_in[p * 1024 : (p + 1) * 1024, :]],
                        outs=[rs_out[p * 256 : (p + 1) * 256, :]],
                    )

            apply_q(0)
            apply_q(1)
            for c in range(NIC):
                den_tn = attention_block(c)
                if c + 2 < NIC:
                    apply_q(c + 2)
                if pending_tail:
                    tail_block(*pending_tail.pop())
                if c >= 2:
                    dw_chunk(c - 2)
                pending_tail.append((c, *den_tn))
            tail_block(*pending_tail.pop())
            dw_chunk(NIC - 2)
            dw_chunk(NIC - 1)

        # ============ Stage D: LayerNorm2d on the scattered shards ============
        with tc.tile_pool(name="stageD", bufs=2) as pD:
            w_b = pD.tile([128, DIM], dt.float32, bufs=1)
            b_b = pD.tile([128, DIM], dt.float32, bufs=1)
            nc.sync.dma_start(out=w_b, in_=ln2w[:, :].to_broadcast([128, DIM]))
            nc.sync.dma_start(out=b_b, in_=ln2b[:, :].to_broadcast([128, DIM]))
            for q2 in range(4):  # one DMA round-trip per ReduceScatter quarter
                R = pD.tile([128, 2, DIM], dt.bfloat16, tag="Rb")
                nc.sync.dma_start(
                    out=R,
                    in_=rs_out[q2 * 256 : (q2 + 1) * 256, :].rearrange(
                        "(j p) c -> p j c", p=128
                    ),
                )
                Rf = pD.tile([128, 2, DIM], dt.float32, tag="R")
                nc.vector.tensor_copy(out=Rf, in_=R)
                R2 = pD.tile([128, 2, DIM], dt.float32, tag="R2")
                for j in range(2):
                    st = pD.tile([128, 6], dt.float32, tag="st")
                    nc.vector.bn_stats(out=st, in_=Rf[:, j, :])
                    mv = pD.tile([128, 2], dt.float32, tag="mv")
                    nc.vector.bn_aggr(out=mv, in_=st)
                    sd = pD.tile([128, 1], dt.float32, tag="sd")
                    nc.scalar.activation(out=sd, in_=mv[:, 1:2], func=AF.Ln, bias=epsP)
                    nc.scalar.activation(out=sd, in_=sd, func=AF.Exp, scale=-0.5)
                    nc.vector.tensor_scalar(
                        out=Rf[:, j, :],
                        in0=Rf[:, j, :],
                        scalar1=mv[:, 0:1],
                        scalar2=sd,
                        op0=OP.subtract,
                        op1=OP.mult,
                    )
                    nc.vector.tensor_mul(out=R2[:, j, :], in0=Rf[:, j, :], in1=w_b)
                    nc.vector.tensor_add(out=R2[:, j, :], in0=R2[:, j, :], in1=b_b)
                nc.sync.dma_start(
                    out=out_ext[q2 * 256 : (q2 + 1) * 256, :].rearrange(
                        "(j p) c -> p j c", p=128
                    ),
                    in_=R2,
                )

    return nc


_cached = {}


def _get_nc():
    if "nc" not in _cached:
        nc = _build()
        nc.finalize()
        _cached["nc"] = nc
    return _cached["nc"]


def _make_in_maps(inputs):
    import ml_dtypes

    bf = ml_dtypes.bfloat16
    x = np.asarray(inputs["x"], np.float32)
    pe_w = np.asarray(inputs["pe_w"], np.float32)
    pe_b = np.asarray(inputs["pe_b"], np.float32)
    qkv_dw = np.asarray(inputs["qkv_dw"], np.float32)
    qkv_pw = np.asarray(inputs["qkv_pw"], np.float32)
    out_dw = np.asarray(inputs["out_dw"], np.float32)
    out_pw = np.asarray(inputs["out_pw"], np.float32)
    nq_w, nq_b = np.asarray(inputs["nq_w"], np.float32), np.asarray(
        inputs["nq_b"], np.float32
    )
    nk_w, nk_b = np.asarray(inputs["nk_w"], np.float32), np.asarray(
        inputs["nk_b"], np.float32
    )
    no_w, no_b = np.asarray(inputs["no_w"], np.float32), np.asarray(
        inputs["no_b"], np.float32
    )
    ln_w, ln_b = np.asarray(inputs["ln_w"], np.float32), np.asarray(
        inputs["ln_b"], np.float32
    )

    gx = np.linspace(0.0, 1.0, Hs, dtype=np.float64)
    gy = np.linspace(0.0, 1.0, Ws, dtype=np.float64)
    pos = (
        pe_w[:, 0:1, None] * gx[None, :, None]
        + pe_w[:, 1:2, None] * gy[None, None, :]
        + pe_b[:, None, None]
    )  # [DIM, H, W]
    posp = np.pad(pos, ((0, 0), (1, 1), (1, 1)))
    taps9 = qkv_dw.reshape(DIM, 9)
    dwpos = np.zeros((DIM, Hs, Ws), np.float64)
    t = 0
    for di in range(3):
        for dj in range(3):
            dwpos += posp[:, di : di + Hs, dj : dj + Ws] * taps9[:, t][:, None, None]
            t += 1
    dwpos = dwpos.reshape(DIM, N).astype(bf)

    idx = np.arange(DH)
    qdiags = np.zeros((DIM, 9, DIM), np.float32)
    taps = qkv_dw.reshape(DIM, 9)
    for t in range(9):
        qdiags[np.arange(DIM), t, np.arange(DIM)] = taps[:, t]
    o8sel = np.zeros((DH, 8, 8), np.float32)
    for c in range(8):
        o8sel[:, c, c] = 1.0 / DH

    in_maps = []
    for core in range(8):
        b, h = core // 4, core % 4
        rows = h + HEADS * idx
        odiags = np.zeros((DH, 9, DH), np.float32)
        otaps = out_dw[rows].reshape(DH, 9)
        for t in range(9):
            odiags[idx, t, idx] = otaps[:, t]
        w8q = np.zeros((8, 8, DH), np.float32)
        w8k = np.zeros((8, 8, DH), np.float32)
        for c in range(8):
            w8q[c, c, :] = nq_w[h]
            w8k[c, c, :] = nk_w[h]
        xg = np.zeros((DIM, GR, GC), bf)
        xg[:, 1 : 1 + Hs, 2 : 2 + Ws] = x[b].reshape(DIM, Hs, Ws).astype(bf)
        m = {
            "x": np.ascontiguousarray(xg.reshape(DIM, GR * GC)),
            "dwpos": dwpos,
            "qdiags": np.ascontiguousarray(qdiags.reshape(DIM, 9 * DIM)).astype(bf),
            "pwq": np.ascontiguousarray(qkv_pw[rows, :].T).astype(bf),
            "pwk": np.ascontiguousarray(qkv_pw[DIM * 2 + rows, :].T).astype(bf),
            "pwv": np.ascontiguousarray(qkv_pw[DIM * 4 + rows, :].T).astype(bf),
            "o8sel": np.ascontiguousarray(o8sel.reshape(DH, 64)).astype(bf),
            "o8self": np.ascontiguousarray(o8sel.reshape(DH, 64)),
            "w8q": np.ascontiguousarray(w8q.transpose(1, 0, 2).reshape(8, 8 * DH)),
            "w8k": np.ascontiguousarray(w8k.transpose(1, 0, 2).reshape(8, 8 * DH)),
            "lnqb": np.ascontiguousarray(nq_b[h][:, None]),
            "lnkb": np.ascontiguousarray(nk_b[h][:, None]),
            "lnow": np.ascontiguousarray(no_w[h][None, :]),
            "lnob": np.ascontiguousarray(no_b[h][:, None]),
            "odiags": np.ascontiguousarray(odiags.reshape(DH, 9 * DH)).astype(bf),
            "opw": np.ascontiguousarray(out_pw[:, rows].T).astype(bf),
            "ln2w": np.ascontiguousarray(ln_w[None, :]),
            "ln2b": np.ascontiguousarray(ln_b[None, :]),
            "o64h": np.full((DH, 1), 1.0 / DH, np.float32),
            "onesr": np.ones((1, DH), np.float32),
        }
        in_maps.append(m)
    return in_maps


def run_on_device(inputs, **kw):
    nc = _get_nc()
    in_maps = _make_in_maps(inputs)
    res = run_bass_kernel_spmd(nc, in_maps, core_ids=list(range(8)), **kw)
    out = np.zeros((B, DIM, N), np.float32)
    for core in range(8):
        b, h = core // 4, core % 4
        o = res.results[core]["out"]  # rows: 4 parts x 256 positions
        for p in range(4):
            g0 = p * 1024 + h * 256
            out[b][:, g0 : g0 + 256] = o[p * 256 : (p + 1) * 256].T
    return out.reshape(B, DIM, Hs, Ws), res


def kernel(**inputs):
    out, _ = run_on_device(inputs)
    return out


# revision 50
# speedup vs baseline: 1.1493x; 1.1493x over previous
"""Trainium2 Bass kernel for nn_Attention_19404662243470.

Sharding: 8 cores = (batch 2) x (heads 4). Each core computes the full
attention pipeline for its (b, h) pair in transposed layout [d, n]; the
final pointwise conv partials are ReduceScattered (2 ops) within each
batch's 4-core group, and LayerNorm2d runs on each core's position shard.

Key layout/speed choices:
 - everything bf16 on the PE paths (FWL weight loads, halved DMA bytes);
   fp32 only inside PSUM and the LN scalar chains.
 - q/k/v come out of the pointwise conv directly as [d, n] ("T layout").
 - softmax runs without max-subtraction; denominator falls out of the AV
   matmul via an appended ones-row in V.
 - exp is split across engines: ACT does 6 of 8 j-groups per chunk,
   DVE does 2 via the Schraudolph int-trick (x*A+B -> int32, whose top
   16 bits ARE the bf16 exp); the AV matmul reads the int32 tile through
   a stride-2 bf16 view.
 - out-LN uses scale invariance: LN(num/den + v) == LN(num + den*v), so
   no reciprocal is ever computed.
 - per-head LN stats in stage A are batched: chunk c's mean/E[x^2] rows
   land on PSUM partition c via a column-selecting stationary, so the
   whole LN chain runs on [8, 512] tiles instead of [1, 4096].
 - q/k halves are mirrored to partitions 64-127 with SBUF->SBUF DMA
   (row packing doubles S^T throughput; contraction is only 64 deep).
"""

import numpy as np

import concourse.bass as bass
import concourse.tile as tile
from concourse import bacc, mybir
from concourse.bass_utils import run_bass_kernel_spmd

dt = mybir.dt
AF = mybir.ActivationFunctionType
OP = mybir.AluOpType

B, DIM, Hs, Ws = 2, 128, 64, 64
HEADS, DH = 4, 64
N = Hs * Ws  # 4096
EPS = 1e-6
IC = 512  # i-chunk width
NIC = N // IC  # 8
JB = 128  # j-block
NJB = N // JB  # 32
NCH = N // 128  # 32
GR = Hs + 2  # 66 grid rows
GC = Hs + 4  # 68 grid cols (interior at col 2 -> 4B-aligned bf16 rows)

# Schraudolph fast-exp: exp(x/8) ~= bf16_bits(int32(x*EXA + EXB) >> 16)
EXA = float((2.0**23) / np.log(2.0) / 8.0)
EXB = float(127 * 2**23 - 335000.0)

_TABLES_PATCHED = False


def _patch_act_tables():
    """Restrict Exp/Ln to the natural_log_exp_and_others set so the ACT
    table never reloads between the softmax Exp stream and the LN-chain
    Ln/Exp pairs (a reload costs ~2.7us and stalls the exp feed)."""
    global _TABLES_PATCHED
    if _TABLES_PATCHED:
        return
    from concourse import bacc as _bacc_mod

    orig = _bacc_mod.get_activation_tables

    def patched(arch):
        tabs = dict(orig(arch))
        keep = {mybir.ActivationFunctionType.Exp, mybir.ActivationFunctionType.Ln}
        return {
            name: (fns if name == "natural_log_exp_and_others" else fns - keep)
            for name, fns in tabs.items()
        }

    _bacc_mod.get_activation_tables = patched
    _TABLES_PATCHED = True


def _build():
    _patch_act_tables()
    nc = bacc.Bacc()

    def par(name, shape, dtyp=dt.float32):
        return nc.declare_dram_parameter(name, list(shape), dtyp, isOutput=False)

    x = par("x", [DIM, GR * GC], dt.bfloat16)  # pre-padded grid layout
    dwpos = par("dwpos", [DIM, N], dt.bfloat16)  # dw3x3(pos), CPU-precomputed
    qdiags = par("qdiags", [DIM, 9 * DIM], dt.bfloat16)
    pwq = par("pwq", [DIM, DH], dt.bfloat16)
    pwk = par("pwk", [DIM, DH], dt.bfloat16)
    pwv = par("pwv", [DIM, DH], dt.bfloat16)
    o8sel = par("o8sel", [DH, 8 * 8], dt.bfloat16)  # slice c: [64,8], col c=1/64
    o8self = par("o8self", [DH, 8 * 8], dt.float32r)  # f32r twin (for f32r moving)
    w8q = par("w8q", [8, 8 * DH], dt.float32r)  # slice c: [8,64], row c=nq_w
    w8k = par("w8k", [8, 8 * DH], dt.float32r)
    lnqb = par("lnqb", [DH, 1])
    lnkb = par("lnkb", [DH, 1])
    lnow = par("lnow", [1, DH], dt.float32r)
    lnob = par("lnob", [DH, 1])
    odiags = par("odiags", [DH, 9 * DH], dt.bfloat16)
    opw = par("opw", [DH, DIM], dt.bfloat16)
    ln2w = par("ln2w", [1, DIM])
    ln2b = par("ln2b", [1, DIM])
    o64hd = par("o64h", [DH, 1], dt.float32r)
    onesrd = par("onesr", [1, DH], dt.float32r)
    out_ext = nc.declare_dram_parameter("out", [N // 4, DIM], dt.float32, isOutput=True)

    rs_in = nc.dram_tensor("rs_in", [N, DIM], dt.bfloat16)
    rs_out = nc.dram_tensor("rs_out", [N // 4, DIM], dt.bfloat16)

    with (
        nc.allow_low_precision(reason="bf16 compute by design"),
        tile.TileContext(nc) as tc,
        tc.tile_pool(name="main", bufs=1) as main,
        tc.tile_pool(name="tmp2", bufs=2) as tmp2,
    ):
        # ---- input DMAs first: they gate the depthwise conv ----
        Xg = main.tile([DIM, GR * GC], dt.bfloat16)
        DWPOS = main.tile([DIM, N], dt.bfloat16)
        qdg = main.tile([DIM, 9, DIM], dt.bfloat16)
        HGC = (GR // 2) * GC
        nc.sync.dma_start(out=Xg[:, 0:HGC], in_=x[:, 0:HGC])
        nc.scalar.dma_start(out=Xg[:, HGC:], in_=x[:, HGC:])
        nc.gpsimd.dma_start(
            out=qdg, in_=qdiags[:, :].rearrange("p (t c) -> p t c", t=9)
        )
        nc.gpsimd.dma_start(out=DWPOS, in_=dwpos[:, :])
        Xg = Xg.rearrange("p (r c) -> p r c", c=GC)

        # ---- persistent SBUF tiles ----
        QL = main.tile([128, N], dt.bfloat16)  # LN'd q, duplicated on both halves
        KL = main.tile([128, N], dt.bfloat16)
        QRW = main.tile([DH, N], dt.bfloat16)  # raw q (pre-LN); applied per-chunk
        SCB8q = main.tile([8, 2 * IC], dt.float32r)  # q-LN rs | mu*rs
        VT = main.tile([DH, N], dt.bfloat16)  # v^T for the skip connection
        V = main.tile([128, NCH, DH + 1], dt.bfloat16)
        SC = main.tile([1, 2 * N], dt.float32)  # attention out-LN: mu | E2
        SCB = main.tile([1, 2 * N], dt.float32r)  # rs | mu*rs (matmul-ready)
        Og = main.tile([DH, GR, GC], dt.bfloat16)  # padded out-LN grid
        odg = main.tile([DH, 9, DH], dt.bfloat16)
        nc.scalar.dma_start(
            out=odg, in_=odiags[:, :].rearrange("p (t c) -> p t c", t=9)
        )
        opw_t = main.tile([DH, DIM], dt.bfloat16)
        nc.scalar.dma_start(out=opw_t, in_=opw[:, :])
        DWO = main.tile([DH, N], dt.bfloat16)
        o64h = main.tile([DH, 1], dt.float32r)
        nc.sync.dma_start(out=o64h, in_=o64hd[:, :])
        o8sel_t = main.tile([DH, 8, 8], dt.bfloat16)
        nc.sync.dma_start(out=o8sel_t, in_=o8sel[:, :].rearrange("p (c e) -> p c e", c=8))
        o8self_t = main.tile([DH, 8, 8], dt.float32r)
        nc.sync.dma_start(
            out=o8self_t, in_=o8self[:, :].rearrange("p (c e) -> p c e", c=8)
        )
        w8q_t = main.tile([8, 8, DH], dt.float32r)
        nc.sync.dma_start(out=w8q_t, in_=w8q[:, :].rearrange("p (c e) -> p c e", c=8))
        w8k_t = main.tile([8, 8, DH], dt.float32r)
        nc.sync.dma_start(out=w8k_t, in_=w8k[:, :].rearrange("p (c e) -> p c e", c=8))
        lnqb_t = main.tile([DH, 1], dt.float32)
        lnkb_t = main.tile([DH, 1], dt.float32)
        lnob_t = main.tile([DH, 1], dt.float32)
        nc.sync.dma_start(out=lnqb_t, in_=lnqb[:, :])
        nc.sync.dma_start(out=lnkb_t, in_=lnkb[:, :])
        nc.sync.dma_start(out=lnob_t, in_=lnob[:, :])
        lnow_t = main.tile([1, DH], dt.float32r)
        nc.sync.dma_start(out=lnow_t, in_=lnow[:, :])
        onesr = main.tile([1, DH], dt.float32r)
        nc.sync.dma_start(out=onesr, in_=onesrd[:, :])
        epsP = main.tile([128, 1], dt.float32)
        nc.vector.memset(epsP, EPS)
        nc.vector.memset(V, 1.0)
        nc.vector.memset(Og, 0.0)

        # ============ Stage A1: pos + depthwise ============
        with tc.tile_pool(name="stageA", bufs=1) as pA:
            psA1cm = tc.tile_pool(name="psA1", bufs=2, space="PSUM")
            psA1 = psA1cm.__enter__()
            pADWcm = tc.tile_pool(name="pADW", bufs=1)
            pADW = pADWcm.__enter__()
            pwq_t = pA.tile([DIM, DH], dt.bfloat16)
            pwk_t = pA.tile([DIM, DH], dt.bfloat16)
            pwv_t = pA.tile([DIM, DH], dt.bfloat16)
            nc.sync.dma_start(out=pwq_t, in_=pwq[:, :])
            nc.sync.dma_start(out=pwk_t, in_=pwk[:, :])
            nc.sync.dma_start(out=pwv_t, in_=pwv[:, :])

            # depthwise 3x3 via 9 accumulated diag matmuls (+ dw(pos) bias)
            Yr = pA.tile([DIM, N], dt.bfloat16)
            for c in range(NIC):
                dwp = psA1.tile([DIM, IC], dt.float32, tag="dw")
                r0 = c * 8
                t = 0
                for di in range(3):
                    for dj in range(3):
                        nc.tensor.matmul(
                            dwp,
                            qdg[:, t, :],
                            Xg[:, r0 + di : r0 + di + 8, 1 + dj : 1 + dj + Ws],
                            start=(t == 0),
                            stop=(t == 8),
                        )
                        t += 1
                nc.vector.tensor_add(
                    out=Yr[:, c * IC : (c + 1) * IC],
                    in0=DWPOS[:, c * IC : (c + 1) * IC],
                    in1=dwp,
                )

            psA1cm.__exit__(None, None, None)
            pADWcm.__exit__(None, None, None)
            # ============ Stage A2: pointwise + q/k LN + v ============
            with tc.tile_pool(name="psA2", bufs=1, space="PSUM") as psA2:
                KRAW = pA.tile([DH, N], dt.bfloat16)
                SC8q = pA.tile([8, 2 * IC], dt.float32)
                SC8k = pA.tile([8, 2 * IC], dt.float32)
                SCB8k = pA.tile([8, 2 * IC], dt.float32r)

                def ptwise(dst, pw_t):
                    for c in range(NIC):
                        qp = psA2.tile([DH, IC], dt.float32, tag="qp", bufs=2)
                        nc.tensor.matmul(
                            qp, pw_t, Yr[:, c * IC : (c + 1) * IC], start=True, stop=True
                        )
                        # alternate copy engines: ACT and DVE drain PSUM in
                        # parallel so the stats matmuls are gated half as long
                        if c % 2 == 0:
                            nc.scalar.copy(out=dst[:, c * IC : (c + 1) * IC], in_=qp)
                        else:
                            nc.vector.tensor_copy(
                                out=dst[:, c * IC : (c + 1) * IC], in_=qp
                            )

                def stats8(src, sc8):
                    """mu and E[x^2] of every chunk, chunk c on partition c."""
                    mu8 = psA2.tile([8, IC], dt.float32, tag="mu8", bufs=1)
                    e28 = psA2.tile([8, IC], dt.float32, tag="e28", bufs=1)
                    for c in range(NIC):
                        sq = tmp2.tile([DH, IC], dt.float32r, tag="sq", bufs=2)
                        src_c = src[:, c * IC : (c + 1) * IC]
                        nc.vector.tensor_mul(out=sq, in0=src_c, in1=src_c)
                        nc.tensor.matmul(
                            mu8,
                            o8sel_t[:, c, :],
                            src_c,
                            start=(c == 0),
                            stop=(c == NIC - 1),
                            skip_group_check=True,
                        )
                        nc.tensor.matmul(
                            e28,
                            o8self_t[:, c, :],
                            sq,
                            start=(c == 0),
                            stop=(c == NIC - 1),
                            skip_group_check=True,
                        )
                    nc.scalar.copy(out=sc8[:, 0:IC], in_=mu8)
                    nc.scalar.copy(out=sc8[:, IC : 2 * IC], in_=e28)

                def chain8(sc8, scb8):
                    mu = sc8[:, 0:IC]
                    e2 = sc8[:, IC : 2 * IC]
                    rs = scb8[:, 0:IC]
                    mrs = scb8[:, IC : 2 * IC]
                    nc.vector.scalar_tensor_tensor(
                        out=mrs, in0=mu, scalar=-1.0, in1=mu, op0=OP.mult, op1=OP.mult
                    )
                    nc.vector.tensor_add(out=e2, in0=e2, in1=mrs)
                    nc.scalar.activation(out=e2, in_=e2, func=AF.Ln, bias=epsP[0:8, :])
                    nc.scalar.activation(out=rs, in_=e2, func=AF.Exp, scale=-0.5)
                    nc.vector.tensor_mul(out=mrs, in0=mu, in1=rs)

                def apply8(src, scb8, w8_t, b_t, dst):
                    for c in range(NIC):
                        bcA = psA2.tile([DH, IC], dt.float32, tag="bc", bufs=2)
                        nc.tensor.matmul(
                            bcA, w8_t[:, c, :], scb8[:, 0:IC], start=True, stop=True
                        )
                        bcB = psA2.tile([DH, IC], dt.float32, tag="bc", bufs=2)
                        nc.tensor.matmul(
                            bcB,
                            w8_t[:, c, :],
                            scb8[:, IC : 2 * IC],
                            start=True,
                            stop=True,
                        )
                        T = tmp2.tile([DH, IC], dt.bfloat16, tag="T")
                        nc.vector.tensor_mul(
                            out=T, in0=src[:, c * IC : (c + 1) * IC], in1=bcA
                        )
                        nc.vector.scalar_tensor_tensor(
                            out=dst[0:DH, c * IC : (c + 1) * IC],
                            in0=T,
                            scalar=b_t,
                            in1=bcB,
                            op0=OP.add,
                            op1=OP.subtract,
                        )

                def vbuild(lo, hi):
                    """V ([pos, dh] layout) + VT ([dh, pos]) for chunks [lo, hi);
                    pure-PE filler issued under the LN chains to keep HAM warm."""
                    for g in range(lo, hi):
                        vp = psA2.tile([128, 4 * DH], dt.float32, tag="vp", bufs=2)
                        for t in range(4):
                            ch = 4 * g + t
                            nc.tensor.matmul(
                                vp[:, t * DH : (t + 1) * DH],
                                Yr[:, ch * 128 : (ch + 1) * 128],
                                pwv_t,
                                start=True,
                                stop=True,
                            )
                        if g % 2 == 0:
                            nc.scalar.copy(
                                out=V[:, 4 * g : 4 * g + 4, 0:DH],
                                in_=vp.rearrange("p (t d) -> p t d", t=4),
                            )
                        else:
                            nc.vector.tensor_copy(
                                out=V[:, 4 * g : 4 * g + 4, 0:DH],
                                in_=vp.rearrange("p (t d) -> p t d", t=4),
                            )
                        qp = psA2.tile([DH, IC], dt.float32, tag="qp", bufs=2)
                        nc.tensor.matmul(
                            qp, pwv_t, Yr[:, g * IC : (g + 1) * IC], start=True, stop=True
                        )
                        if g % 2 == 1:
                            nc.scalar.copy(out=VT[:, g * IC : (g + 1) * IC], in_=qp)
                        else:
                            nc.vector.tensor_copy(
                                out=VT[:, g * IC : (g + 1) * IC], in_=qp
                            )

                # k first; its chain overlaps v-builds + q pointwise on the PE.
                # q's LN apply is deferred into the attention loop (chunk-wise)
                # so the PE never idles long enough for HAM to re-throttle.
                ptwise(KRAW, pwk_t)
                stats8(KRAW, SC8k)
                chain8(SC8k, SCB8k)
                vbuild(0, NIC // 2)
                ptwise(QRW, pwq_t)
                stats8(QRW, SC8q)

                apply8(KRAW, SCB8k, w8k_t, lnkb_t, KL)
                nc.scalar.dma_start(out=KL[DH:128, :], in_=KL[0:DH, :])

                chain8(SC8q, SCB8q)
                vbuild(NIC // 2, NIC)

        # ============ Stage B: attention with inline out-LN ============
        with tc.tile_pool(name="psB", bufs=1, space="PSUM") as psB, tc.tile_pool(
            name="sbB", bufs=3
        ) as sbB:
            NG = NJB // 2  # 16 pair-groups per chunk
            DVE_G = (3, 7, 11, 15)  # groups whose exp runs on DVE (Schraudolph)
            pending_tail = []

            def apply_q(c):
                """q-LN apply for chunk c, pipelined inside the attention loop."""
                bcA = psB.tile([DH, IC], dt.float32, tag="st", bufs=2)
                nc.tensor.matmul(
                    bcA, w8q_t[:, c, :], SCB8q[:, 0:IC], start=True, stop=True
                )
                bcB = psB.tile([DH, IC], dt.float32, tag="st", bufs=2)
                nc.tensor.matmul(
                    bcB, w8q_t[:, c, :], SCB8q[:, IC : 2 * IC], start=True, stop=True
                )
                T = tmp2.tile([DH, IC], dt.bfloat16, tag="Tq")
                nc.vector.tensor_mul(
                    out=T, in0=QRW[:, c * IC : (c + 1) * IC], in1=bcA
                )
                nc.vector.scalar_tensor_tensor(
                    out=QL[0:DH, c * IC : (c + 1) * IC],
                    in0=T,
                    scalar=lnqb_t,
                    in1=bcB,
                    op0=OP.add,
                    op1=OP.subtract,
                )
                nc.scalar.dma_start(
                    out=QL[DH:128, c * IC : (c + 1) * IC],
                    in_=QL[0:DH, c * IC : (c + 1) * IC],
                )

            def attention_block(c):
                avp = psB.tile([DH + 1, IC], dt.float32, tag="avp", bufs=1)
                stgs = {}
                Es = {}

                def issue_st(g):
                    stg = psB.tile([128, 2 * IC], dt.float32, tag="stg", bufs=2)
                    j0 = 2 * g * JB
                    nc.tensor.matmul(
                        stg[:, 0:IC],
                        KL[0:DH, j0 : j0 + JB],
                        QL[0:DH, c * IC : (c + 1) * IC],
                        start=True,
                        stop=True,
                    )
                    nc.tensor.matmul(
                        stg[:, IC : 2 * IC],
                        KL[DH:128, j0 + JB : j0 + 2 * JB],
                        QL[DH:128, c * IC : (c + 1) * IC],
                        start=True,
                        stop=True,
                    )
                    stgs[g] = stg

                def issue_exp(g):
                    if g in DVE_G:
                        EI = sbB.tile([128, 2 * IC], dt.int32, tag="EI", bufs=2)
                        nc.vector.tensor_scalar(
                            out=EI,
                            in0=stgs.pop(g),
                            scalar1=EXA,
                            scalar2=EXB,
                            op0=OP.mult,
                            op1=OP.add,
                        )
                        Es[g] = EI.bitcast(dt.bfloat16).rearrange(
                            "p (a two) -> p a two", two=2
                        )
                    else:
                        E = sbB.tile([128, 2 * IC], dt.bfloat16, tag="E")
                        nc.scalar.activation(
                            out=E, in_=stgs.pop(g), func=AF.Exp, scale=float(DH**-0.5)
                        )
                        Es[g] = E

                def issue_av(g):
                    E = Es.pop(g)
                    for t in range(2):
                        jb = 2 * g + t
                        if g in DVE_G:
                            rhs = E[:, t * IC : (t + 1) * IC, 1:2]
                        else:
                            rhs = E[:, t * IC : (t + 1) * IC]
                        nc.tensor.matmul(
                            avp,
                            V[:, jb, :],
                            rhs,
                            start=(jb == 0),
                            stop=(jb == NJB - 1),
                            skip_group_check=True,
                        )

                issue_st(0)
                issue_exp(0)
                for g in range(1, NG):
                    issue_st(g)
                    issue_exp(g)
                    issue_av(g - 1)
                issue_av(NG - 1)

                # park numerator+denominator info quickly to free avp:
                # DEN row copy; numerator stays in avp until tail (bufs=1 ok:
                # tail runs during the NEXT chunk's matmuls, before its avp use)
                DEN = sbB.tile([1, IC], dt.float32r, tag="DEN", bufs=2)
                nc.vector.tensor_copy(out=DEN, in_=avp[DH : DH + 1, :])
                Tn = sbB.tile([DH, IC], dt.float32, tag="Tn", bufs=2)
                nc.vector.tensor_copy(out=Tn, in_=avp[0:DH, :])
                return DEN, Tn

            def stats_mms(psp, src_ap, c):
                sq = tmp2.tile([DH, IC], dt.float32r, tag="sqo", bufs=1)
                nc.gpsimd.tensor_mul(out=sq, in0=src_ap, in1=src_ap)
                smu = psp.tile([1, IC], dt.float32, tag="st", bufs=2)
                nc.tensor.matmul(smu, o64h, src_ap, start=True, stop=True)
                nc.vector.tensor_copy(out=SC[:, c * IC : (c + 1) * IC], in_=smu)
                se2 = psp.tile([1, IC], dt.float32, tag="st", bufs=2)
                nc.tensor.matmul(se2, o64h, sq, start=True, stop=True)
                nc.vector.tensor_copy(out=SC[:, N + c * IC : N + (c + 1) * IC], in_=se2)

            def ln_chain(lo, hi):
                mu = SC[:, lo:hi]
                e2 = SC[:, N + lo : N + hi]
                mrs = SCB[:, N + lo : N + hi]
                rs = SCB[:, lo:hi]
                nc.vector.scalar_tensor_tensor(
                    out=mrs, in0=mu, scalar=-1.0, in1=mu, op0=OP.mult, op1=OP.mult
                )
                nc.vector.tensor_add(out=e2, in0=e2, in1=mrs)
                nc.scalar.activation(out=e2, in_=e2, func=AF.Ln, bias=epsP[0:1, :])
                nc.scalar.activation(out=rs, in_=e2, func=AF.Exp, scale=-0.5)
                nc.vector.tensor_mul(out=mrs, in0=mu, in1=rs)

            def tail_block(c, DEN, Tn):
                # scale-invariant skip: OSc = num + den*v (LN output matches
                # LN(num/den + v) because LN normalizes per-position scale)
                bcD = psB.tile([DH, IC], dt.float32, tag="st", bufs=2)
                nc.tensor.matmul(bcD, onesr, DEN, start=True, stop=True)
                OSc = sbB.tile([DH, IC], dt.float32r, tag="OS", bufs=2)
                nc.vector.tensor_mul(
                    out=OSc, in0=VT[:, c * IC : (c + 1) * IC], in1=bcD
                )
                nc.vector.tensor_add(out=OSc, in0=OSc, in1=Tn)
                stats_mms(psB, OSc[:, :], c)
                ln_chain(c * IC, (c + 1) * IC)
                bcA = psB.tile([DH, IC], dt.float32, tag="st", bufs=2)
                nc.tensor.matmul(
                    bcA, lnow_t, SCB[:, c * IC : (c + 1) * IC], start=True, stop=True
                )
                bcB = psB.tile([DH, IC], dt.float32, tag="st", bufs=2)
                nc.tensor.matmul(
                    bcB,
                    lnow_t,
                    SCB[:, N + c * IC : N + (c + 1) * IC],
                    start=True,
                    stop=True,
                )
                T = tmp2.tile([DH, IC], dt.float32, tag="T")
                nc.vector.tensor_mul(out=T, in0=OSc, in1=bcA)
                r0 = c * 8
                nc.vector.scalar_tensor_tensor(
                    out=Og[:, 1 + r0 : 9 + r0, 2 : 2 + Ws],
                    in0=T.rearrange("p (a b) -> p a b", b=Ws),
                    scalar=lnob_t,
                    in1=bcB.rearrange("p (a b) -> p a b", b=Ws),
                    op0=OP.add,
                    op1=OP.subtract,
                )

            def dw_chunk(c):
                dwpf = psB.tile([128, IC], dt.float32, tag="dwpp", bufs=1)
                dwp = dwpf[0:DH, :]
                r0 = c * 8
                t = 0
                for di in range(3):
                    for dj in range(3):
                        nc.tensor.matmul(
                            dwp,
                            odg[:, t, :],
                            Og[:, r0 + di : r0 + di + 8, 1 + dj : 1 + dj + Ws],
                            start=(t == 0),
                            stop=(t == 8),
                        )
                        t += 1
                nc.vector.tensor_copy(out=DWO[:, c * IC : (c + 1) * IC], in_=dwp)
                pp = psB.tile([128, 4 * DIM], dt.float32, tag="dwpp", bufs=1)
                for t in range(4):
                    ch = 4 * c + t
                    nc.tensor.matmul(
                        pp[:, t * DIM : (t + 1) * DIM],
                        DWO[:, ch * 128 : (ch + 1) * 128],
                        opw_t,
                        start=True,
                        stop=True,
                    )
                PP = tmp2.tile([128, 4 * DIM], dt.bfloat16, tag="PP")
                nc.vector.tensor_copy(out=PP, in_=pp)
                for t in range(4):
                    ch = 4 * c + t
                    nc.sync.dma_start(
                        out=rs_in[ch * 128 : (ch + 1) * 128, :],
                        in_=PP[:, t * DIM : (t + 1) * DIM],
                    )
                if c % 2 == 1:
                    p = c // 2
                    nc.gpsimd.collective_compute(
                        "ReduceScatter",
                        OP.add,
                        replica_groups=[[0, 1, 2, 3], [4, 5, 6, 7]],
                        ins=[rs_in[p * 1024 : (p + 1) * 1024, :]],
                        outs=[rs_out[p * 256 : (p + 1) * 256, :]],
                    )

            apply_q(0)
            apply_q(1)
            for c in range(NIC):
                den_tn = attention_block(c)
                if c + 2 < NIC:
                    apply_q(c + 2)
                if pending_tail:
                    tail_block(*pending_tail.pop())
                if c >= 2:
                    dw_chunk(c - 2)
                pending_tail.append((c, *den_tn))
            tail_block(*pending_tail.pop())
            dw_chunk(NIC - 2)
            dw_chunk(NIC - 1)

        # ============ Stage D: LayerNorm2d on the scattered shards ============
        with tc.tile_pool(name="stageD", bufs=2) as pD:
            w_b = pD.tile([128, DIM], dt.float32, bufs=1)
            b_b = pD.tile([128, DIM], dt.float32, bufs=1)
            nc.sync.dma_start(out=w_b, in_=ln2w[:, :].to_broadcast([128, DIM]))
            nc.sync.dma_start(out=b_b, in_=ln2b[:, :].to_broadcast([128, DIM]))
            for q2 in range(4):  # one DMA round-trip per ReduceScatter quarter
                R = pD.tile([128, 2, DIM], dt.bfloat16, tag="Rb")
                nc.sync.dma_start(
                    out=R,
                    in_=rs_out[q2 * 256 : (q2 + 1) * 256, :].rearrange(
                        "(j p) c -> p j c", p=128
                    ),
                )
                Rf = pD.tile([128, 2, DIM], dt.float32, tag="R")
                nc.vector.tensor_copy(out=Rf, in_=R)
                R2 = pD.tile([128, 2, DIM], dt.float32, tag="R2")
                for j in range(2):
                    st = pD.tile([128, 6], dt.float32, tag="st")
                    nc.vector.bn_stats(out=st, in_=Rf[:, j, :])
                    mv = pD.tile([128, 2], dt.float32, tag="mv")
                    nc.vector.bn_aggr(out=mv, in_=st)
                    sd = pD.tile([128, 1], dt.float32, tag="sd")
                    nc.scalar.activation(out=sd, in_=mv[:, 1:2], func=AF.Ln, bias=epsP)
                    nc.scalar.activation(out=sd, in_=sd, func=AF.Exp, scale=-0.5)
                    nc.vector.tensor_scalar(
                        out=Rf[:, j, :],
                        in0=Rf[:, j, :],
                        scalar1=mv[:, 0:1],
                        scalar2=sd,
                        op0=OP.subtract,
                        op1=OP.mult,
                    )
                    nc.vector.tensor_mul(out=R2[:, j, :], in0=Rf[:, j, :], in1=w_b)
                    nc.vector.tensor_add(out=R2[:, j, :], in0=R2[:, j, :], in1=b_b)
                nc.sync.dma_start(
                    out=out_ext[q2 * 256 : (q2 + 1) * 256, :].rearrange(
                        "(j p) c -> p j c", p=128
                    ),
                    in_=R2,
                )

    return nc


_cached = {}


def _get_nc():
    if "nc" not in _cached:
        nc = _build()
        nc.finalize()
        _cached["nc"] = nc
    return _cached["nc"]


def _make_in_maps(inputs):
    import ml_dtypes

    bf = ml_dtypes.bfloat16
    x = np.asarray(inputs["x"], np.float32)
    pe_w = np.asarray(inputs["pe_w"], np.float32)
    pe_b = np.asarray(inputs["pe_b"], np.float32)
    qkv_dw = np.asarray(inputs["qkv_dw"], np.float32)
    qkv_pw = np.asarray(inputs["qkv_pw"], np.float32)
    out_dw = np.asarray(inputs["out_dw"], np.float32)
    out_pw = np.asarray(inputs["out_pw"], np.float32)
    nq_w, nq_b = np.asarray(inputs["nq_w"], np.float32), np.asarray(
        inputs["nq_b"], np.float32
    )
    nk_w, nk_b = np.asarray(inputs["nk_w"], np.float32), np.asarray(
        inputs["nk_b"], np.float32
    )
    no_w, no_b = np.asarray(inputs["no_w"], np.float32), np.asarray(
        inputs["no_b"], np.float32
    )
    ln_w, ln_b = np.asarray(inputs["ln_w"], np.float32), np.asarray(
        inputs["ln_b"], np.float32
    )

    gx = np.linspace(0.0, 1.0, Hs, dtype=np.float64)
    gy = np.linspace(0.0, 1.0, Ws, dtype=np.float64)
    pos = (
        pe_w[:, 0:1, None] * gx[None, :, None]
        + pe_w[:, 1:2, None] * gy[None, None, :]
        + pe_b[:, None, None]
    )  # [DIM, H, W]
    posp = np.pad(pos, ((0, 0), (1, 1), (1, 1)))
    taps9 = qkv_dw.reshape(DIM, 9)
    dwpos = np.zeros((DIM, Hs, Ws), np.float64)
    t = 0
    for di in range(3):
        for dj in range(3):
            dwpos += posp[:, di : di + Hs, dj : dj + Ws] * taps9[:, t][:, None, None]
            t += 1
    dwpos = dwpos.reshape(DIM, N).astype(bf)

    idx = np.arange(DH)
    qdiags = np.zeros((DIM, 9, DIM), np.float32)
    taps = qkv_dw.reshape(DIM, 9)
    for t in range(9):
        qdiags[np.arange(DIM), t, np.arange(DIM)] = taps[:, t]
    o8sel = np.zeros((DH, 8, 8), np.float32)
    for c in range(8):
        o8sel[:, c, c] = 1.0 / DH

    in_maps = []
    for core in range(8):
        b, h = core // 4, core % 4
        rows = h + HEADS * idx
        odiags = np.zeros((DH, 9, DH), np.float32)
        otaps = out_dw[rows].reshape(DH, 9)
        for t in range(9):
            odiags[idx, t, idx] = otaps[:, t]
        w8q = np.zeros((8, 8, DH), np.float32)
        w8k = np.zeros((8, 8, DH), np.float32)
        for c in range(8):
            w8q[c, c, :] = nq_w[h]
            w8k[c, c, :] = nk_w[h]
        xg = np.zeros((DIM, GR, GC), bf)
        xg[:, 1 : 1 + Hs, 2 : 2 + Ws] = x[b].reshape(DIM, Hs, Ws).astype(bf)
        m = {
            "x": np.ascontiguousarray(xg.reshape(DIM, GR * GC)),
            "dwpos": dwpos,
            "qdiags": np.ascontiguousarray(qdiags.reshape(DIM, 9 * DIM)).astype(bf),
            "pwq": np.ascontiguousarray(qkv_pw[rows, :].T).astype(bf),
            "pwk": np.ascontiguousarray(qkv_pw[DIM * 2 + rows, :].T).astype(bf),
            "pwv": np.ascontiguousarray(qkv_pw[DIM * 4 + rows, :].T).astype(bf),
            "o8sel": np.ascontiguousarray(o8sel.reshape(DH, 64)).astype(bf),
            "o8self": np.ascontiguousarray(o8sel.reshape(DH, 64)),
            "w8q": np.ascontiguousarray(w8q.transpose(1, 0, 2).reshape(8, 8 * DH)),
            "w8k": np.ascontiguousarray(w8k.transpose(1, 0, 2).reshape(8, 8 * DH)),
            "lnqb": np.ascontiguousarray(nq_b[h][:, None]),
            "lnkb": np.ascontiguousarray(nk_b[h][:, None]),
            "lnow": np.ascontiguousarray(no_w[h][None, :]),
            "lnob": np.ascontiguousarray(no_b[h][:, None]),
            "odiags": np.ascontiguousarray(odiags.reshape(DH, 9 * DH)).astype(bf),
            "opw": np.ascontiguousarray(out_pw[:, rows].T).astype(bf),
            "ln2w": np.ascontiguousarray(ln_w[None, :]),
            "ln2b": np.ascontiguousarray(ln_b[None, :]),
            "o64h": np.full((DH, 1), 1.0 / DH, np.float32),
            "onesr": np.ones((1, DH), np.float32),
        }
        in_maps.append(m)
    return in_maps


def run_on_device(inputs, **kw):
    nc = _get_nc()
    in_maps = _make_in_maps(inputs)
    res = run_bass_kernel_spmd(nc, in_maps, core_ids=list(range(8)), **kw)
    out = np.zeros((B, DIM, N), np.float32)
    for core in range(8):
        b, h = core // 4, core % 4
        o = res.results[core]["out"]  # rows: 4 parts x 256 positions
        for p in range(4):
            g0 = p * 1024 + h * 256
            out[b][:, g0 : g0 + 256] = o[p * 256 : (p + 1) * 256].T
    return out.reshape(B, DIM, Hs, Ws), res


def kernel(**inputs):
    out, _ = run_on_device(inputs)
    return out
